# revision 1
# baseline (speedup 1.0000x reference)
"""Trainium2 distributed causal attention kernel (8 NeuronCores).

Problem: x[4,2048,1024] -> qkv proj -> 16-head causal attention -> out proj.

Sharding (uniform SPMD graph on all 8 cores):
  core c = (batch b = c//2, head-group g = c%2 of 8 heads).
  Each core: projects q/k/v for its 8 heads over the full 2048 tokens of its
  batch, runs causal flash-style attention (no max subtraction -- scores are
  O(1) for this input distribution), computes the partial output projection
  with its 512 inner dims of w_out, adds b_out/2, then a pairwise
  ReduceScatter(add) over {2b, 2b+1} yields final output token-stripes.
  Host reassembles stripes. No other collectives.

Layouts (all matmuls are layout-natural; x is transposed on the host):
  xT   [1024(dm), 2048(tok)]  f32r   (moving operand of kT/qT proj, stationary of v proj)
  kT,qT [512(inner) as 4x[128], 2048] bf16 (keys/queries transposed, 2 heads per tile)
  v_aug [2048(tok) as 16x[128], 8*65] bf16 (per head: 64 v-cols + ones col -> softmax denom)
  simT psum [128(key), 512(tok)] = k-block^T @ q-chunk   (K=64, heads packed 2x in PE array)
  pT = exp(simT * 0.125) bf16, causal band masks applied multiplicatively
  pv psum [65, 512] accumulates over k-blocks (row 64 = denominator)
  attnoutT bf16 [512(inner), 512(tok)] per chunk -> out-proj psum [128(tok), 512(col)]
"""

import sys

sys.path.insert(0, "/opt/trn_rl_repo")

import numpy as np

B, N, DM = 4, 2048, 1024
H, DH = 16, 64
HG = 8  # heads per core
LI = HG * DH  # local inner = 512
NCORES = 8
CHUNK = 512  # q-chunk tokens
NCHUNK = N // CHUNK  # 4
KB = 128  # k-block size
VW = DH + 1  # v columns per head incl. ones column

_GRAPH = None


def _build_graph(dbg=False):
    from concourse import bacc, bass, mybir, tile

    f32 = mybir.dt.float32
    f32r = mybir.dt.float32r
    bf16 = mybir.dt.bfloat16
    Exp = mybir.ActivationFunctionType.Exp

    nc = bacc.Bacc("TRN2", target_bir_lowering=False, debug=False)

    xT_d = nc.dram_tensor("xT", [DM, N], f32r, kind="ExternalInput")
    wq_d = nc.dram_tensor("wq", [DM, LI], f32r, kind="ExternalInput")
    wk_d = nc.dram_tensor("wk", [DM, LI], f32r, kind="ExternalInput")
    wv_d = nc.dram_tensor("wv", [DM, LI], f32r, kind="ExternalInput")
    wo_d = nc.dram_tensor("wo", [LI, DM], f32, kind="ExternalInput")
    hb_d = nc.dram_tensor("hb", [1, DM], f32, kind="ExternalInput")
    mask_d = nc.dram_tensor("mask", [KB, KB], bf16, kind="ExternalInput")
    out_d = nc.dram_tensor("out", [N // 2, DM], f32, kind="ExternalOutput")
    if dbg:
        dkT = nc.dram_tensor("dkT", [128, N], f32, kind="ExternalOutput")
        dqT = nc.dram_tensor("dqT", [128, N], f32, kind="ExternalOutput")
        dva = nc.dram_tensor("dva", [128, HG * VW], f32, kind="ExternalOutput")
        dpt = nc.dram_tensor("dpt", [128, CHUNK], f32, kind="ExternalOutput")
        dpv = nc.dram_tensor("dpv", [VW, CHUNK], f32, kind="ExternalOutput")
        dao = nc.dram_tensor("dao", [128, CHUNK], f32, kind="ExternalOutput")
        dpd = nc.dram_tensor("dpd", [CHUNK, DM], f32, kind="ExternalOutput")

    RG = [[0, 1], [2, 3], [4, 5], [6, 7]]

    with tile.TileContext(nc) as tc:
        with (
            tc.tile_pool(name="persist", bufs=1) as pers,
            tc.tile_pool(name="projtmp", bufs=1) as ptmp,
            tc.tile_pool(name="wstream", bufs=3) as wstr,
            tc.tile_pool(name="work", bufs=4) as work,
            tc.tile_pool(name="aux", bufs=2) as aux,
            tc.tile_pool(name="mmps", bufs=2, space="PSUM") as mmps,
            tc.tile_pool(name="simps", bufs=4, space="PSUM") as simps,
            tc.tile_pool(name="pvps", bufs=2, space="PSUM") as pvps,
            tc.tile_pool(name="dram", bufs=2, space="DRAM") as dram,
        ):
            # ---- constants / persistent tiles ----
            mask_sb = pers.tile([KB, KB], bf16, tag="mask")
            nc.sync.dma_start(out=mask_sb[:, :], in_=mask_d[:, :])

            ones_bf = pers.tile([1, KB], bf16, tag="ones")
            nc.vector.memset(ones_bf[:, :], 1.0)

            hb_f = aux.tile([1, DM], f32, tag="hbf")
            nc.sync.dma_start(out=hb_f[:, :], in_=hb_d[:, :])
            hb_bf = pers.tile([1, DM], bf16, tag="hbb")
            nc.vector.tensor_copy(hb_bf[:, :], hb_f[:, :])

            wo_bf = []
            for it in range(4):
                wof = aux.tile([128, DM], f32, tag="wof", bufs=1)
                nc.sync.dma_start(out=wof[:, :], in_=wo_d[it * 128 : (it + 1) * 128, :])
                wob = pers.tile([128, DM], bf16, tag=f"wo{it}")
                nc.vector.tensor_copy(wob[:, :], wof[:, :])
                wo_bf.append(wob)

            # ---- phase 1: projections ----
            xT = []
            for d in range(8):
                t = ptmp.tile([128, N], f32r, tag=f"xT{d}")
                for cc in range(4):
                    nc.sync.dma_start(
                        out=t[:, cc * 512 : (cc + 1) * 512],
                        in_=xT_d[d * 128 : (d + 1) * 128, cc * 512 : (cc + 1) * 512],
                    )
                xT.append(t)

            kT = [pers.tile([128, N], bf16, tag=f"kT{i}", name=f"kT{i}") for i in range(4)]
            qT = [pers.tile([128, N], bf16, tag=f"qT{i}", name=f"qT{i}") for i in range(4)]

            for w_d, dst in ((wk_d, kT), (wq_d, qT)):
                for it in range(4):
                    wt = []
                    for d in range(8):
                        t = wstr.tile([128, 128], f32r, tag="wt", bufs=8)
                        nc.sync.dma_start(
                            out=t[:, :],
                            in_=w_d[d * 128 : (d + 1) * 128, it * 128 : (it + 1) * 128],
                        )
                        wt.append(t)
                    for tt in range(4):
                        ps = mmps.tile([128, 512], f32, tag="mm")
                        for d in range(8):
                            nc.tensor.matmul(
                                ps[:, :],
                                lhsT=wt[d][:, :],
                                rhs=xT[d][:, tt * 512 : (tt + 1) * 512],
                                start=(d == 0),
                                stop=(d == 7),
                            )
                        nc.vector.tensor_copy(
                            dst[it][:, tt * 512 : (tt + 1) * 512], ps[:, :]
                        )

            wv = []
            for d in range(8):
                t = ptmp.tile([128, LI], f32r, tag=f"wv{d}")
                nc.sync.dma_start(out=t[:, :], in_=wv_d[d * 128 : (d + 1) * 128, :])
                wv.append(t)

            v_aug = [pers.tile([128, HG * VW], bf16, tag=f"va{t}", name=f"va{t}") for t in range(16)]
            for tt in range(16):
                va3 = v_aug[tt].rearrange("p (h c) -> p h c", h=HG)
                nc.vector.memset(va3[:, :, DH : DH + 1], 1.0)
                ps = mmps.tile([128, 512], f32, tag="mm")
                for d in range(8):
                    nc.tensor.matmul(
                        ps[:, :],
                        lhsT=xT[d][:, tt * 128 : (tt + 1) * 128],
                        rhs=wv[d][:, :],
                        start=(d == 0),
                        stop=(d == 7),
                    )
                nc.vector.tensor_copy(
                    va3[:, :, 0:DH], ps.rearrange("p (h c) -> p h c", h=HG)
                )

            if dbg:
                for src_t, dst_d in ((kT[0], dkT), (qT[0], dqT)):
                    for pc in range(4):
                        dc = aux.tile([128, 512], f32, tag="dbgc", bufs=1)
                        nc.vector.tensor_copy(dc[:, :], src_t[:, pc * 512 : (pc + 1) * 512])
                        nc.sync.dma_start(out=dst_d[:, pc * 512 : (pc + 1) * 512], in_=dc[:, :])
                dc = aux.tile([128, HG * VW], f32, tag="dbgc2", bufs=1)
                nc.vector.tensor_copy(dc[:, :], v_aug[0][:, :])
                nc.sync.dma_start(out=dva[:, :], in_=dc[:, :])

            # ---- phases 2+3: attention + out-proj + RS, chunk-pipelined ----
            # outproj of chunk c is emitted AFTER attention of chunk c+1 so the
            # PE FIFO never stalls on c's normalize epilogue (DVE/DMA chain).
            chunk_aos = {}

            def attention_chunk(c):
                nk = 4 * (c + 1)
                aos = [
                    work.tile(
                        [128, CHUNK], bf16, tag=f"ao{i}", name=f"ao{i}", bufs=2
                    )
                    for i in range(4)
                ]
                dn = work.tile([8, CHUNK], f32, tag="dn", name="dn", bufs=2)
                vals = [None] * 8
                for hp in range(4):
                    pvs = [
                        pvps.tile([VW, CHUNK], f32, tag="pv", name="pv")
                        for _ in range(2)
                    ]
                    sims_of = {}

                    def qk_step(jb):
                        sims = [
                            simps.tile([128, CHUNK], f32, tag="sim", name="sim")
                            for _ in range(2)
                        ]
                        for e in range(2):
                            nc.tensor.matmul(
                                sims[e][:, :],
                                lhsT=kT[hp][
                                    64 * e : 64 * e + 64, jb * KB : (jb + 1) * KB
                                ],
                                rhs=qT[hp][
                                    64 * e : 64 * e + 64, c * CHUNK : (c + 1) * CHUNK
                                ],
                                start=True,
                                stop=True,
                            )
                        sims_of[jb] = sims

                    def pv_step(jb):
                        sims = sims_of.pop(jb)
                        v = jb - (nk - 4)
                        col0 = max(0, v) * KB
                        for e in range(2):
                            h = 2 * hp + e
                            pt = work.tile([128, CHUNK], bf16, tag=f"pt{e}", bufs=3)
                            if col0 > 0:
                                nc.vector.memset(pt[:, 0:col0], 0.0)
                            nc.scalar.activation(
                                pt[:, col0:CHUNK],
                                sims[e][:, col0:CHUNK],
                                Exp,
                                scale=float(DH**-0.5),
                            )
                            if v >= 0:
                                nc.vector.tensor_mul(
                                    pt[:, col0 : col0 + KB],
                                    pt[:, col0 : col0 + KB],
                                    mask_sb[:, :],
                                )
                            nc.tensor.matmul(
                                pvs[e][:, :],
                                lhsT=v_aug[jb][:, h * VW : (h + 1) * VW],
                                rhs=pt[:, :],
                                start=(jb == 0),
                                stop=(jb == nk - 1),
                            )

                    qk_step(0)
                    for jb in range(1, nk):
                        qk_step(jb)
                        pv_step(jb - 1)
                    pv_step(nk - 1)

                    # evacuate PV psum to SBUF immediately so the psum banks
                    # free for the next head-pair; denominators collect into dn
                    for e in range(2):
                        h = 2 * hp + e
                        t = work.tile(
                            [DH, CHUNK], bf16, tag=f"pvsb{h}", bufs=2, name=f"pvsb{h}"
                        )
                        nc.vector.tensor_copy(t[:, :], pvs[e][0:DH, :])
                        vals[h] = t
                        tmpd = work.tile([1, CHUNK], f32, tag="tmpd", bufs=2)
                        nc.vector.tensor_copy(tmpd[:, :], pvs[e][DH : DH + 1, :])
                        nc.sync.dma_start(out=dn[h : h + 1, :], in_=tmpd[:, :])

                chunk_aos[c] = (aos, vals, dn)

            def epilogue_chunk(c):
                aos, vals, dn = chunk_aos[c]
                rc = work.tile([8, CHUNK], f32, tag="rc", name="rc", bufs=2)
                nc.vector.reciprocal(rc[:, :], dn[:, :])
                rcb = work.tile([8, CHUNK], bf16, tag="rcb", name="rcb", bufs=2)
                nc.vector.tensor_copy(rcb[:, :], rc[:, :])
                for h in range(8):
                    rb = work.tile([64, CHUNK], bf16, tag="rb", bufs=4)
                    rrow = rcb[h : h + 1, :]
                    rsrc = bass.AP(
                        tensor=rrow.tensor,
                        offset=rrow.offset,
                        ap=[[CHUNK, 1], [0, 64], [1, CHUNK]],
                    )
                    nc.sync.dma_start(out=rb[:, :], in_=rsrc)
                    nc.vector.tensor_mul(
                        aos[h // 2][64 * (h % 2) : 64 * (h % 2) + 64, :],
                        vals[h][:, :],
                        rb[:, :],
                    )

            def outproj_chunk(c, n_rs=1):
                aos, _, _ = chunk_aos.pop(c)
                pd = dram.tile([CHUNK, DM], f32, tag="pd")
                ts_per_rs = 4 // n_rs
                for rs_i in range(n_rs):
                    for ts in range(rs_i * ts_per_rs, (rs_i + 1) * ts_per_rs):
                        for ct in range(2):
                            po = mmps.tile([128, 512], f32, tag="mm")
                            nc.tensor.matmul(
                                po[:, :],
                                lhsT=ones_bf[:, :],
                                rhs=hb_bf[:, ct * 512 : (ct + 1) * 512],
                                start=True,
                                stop=False,
                            )
                            for it in range(4):
                                nc.tensor.matmul(
                                    po[:, :],
                                    lhsT=aos[it][:, ts * 128 : (ts + 1) * 128],
                                    rhs=wo_bf[it][:, ct * 512 : (ct + 1) * 512],
                                    start=False,
                                    stop=(it == 3),
                                )
                            ob = work.tile([128, 512], f32, tag="ob", name="ob", bufs=2)
                            nc.vector.tensor_copy(ob[:, :], po[:, :])
                            nc.sync.dma_start(
                                out=pd[
                                    ts * 128 : (ts + 1) * 128,
                                    ct * 512 : (ct + 1) * 512,
                                ],
                                in_=ob[:, :],
                            )
                    rows = CHUNK // n_rs
                    rs = dram.tile(
                        [rows // 2, DM], f32, tag="rs", name="rs", padded_shape=[CHUNK // 2, DM]
                    )
                    nc.gpsimd.collective_compute(
                        "ReduceScatter",
                        mybir.AluOpType.add,
                        replica_groups=RG,
                        ins=[pd[rs_i * rows : (rs_i + 1) * rows, :].opt()],
                        outs=[rs[:, :].opt()],
                    )
                    out_r0 = c * 256 + rs_i * (rows // 2)
                    nc.sync.dma_start(
                        out=out_d[out_r0 : out_r0 + rows // 2, :], in_=rs[:, :]
                    )

            attention_chunk(0)
            for c in range(1, NCHUNK):
                epilogue_chunk(c - 1)
                attention_chunk(c)
                outproj_chunk(c - 1)
            epilogue_chunk(NCHUNK - 1)
            outproj_chunk(NCHUNK - 1, n_rs=2)

    nc.finalize()
    return nc


def _get_graph():
    global _GRAPH
    if _GRAPH is None:
        _GRAPH = _build_graph()
    return _GRAPH


def _build_masks():
    # [j, ti] = 1 where ti >= j: token ti attends key j within the diagonal block
    return np.ascontiguousarray(np.triu(np.ones((KB, KB), np.float32)))


def _make_in_maps(x, w_qkv, w_out, b_out):
    x = np.asarray(x, np.float32)
    w_qkv = np.asarray(w_qkv, np.float32)
    w_out = np.asarray(w_out, np.float32)
    b_out = np.asarray(b_out, np.float32)
    import ml_dtypes

    xT = [np.ascontiguousarray(x[b].T) for b in range(B)]
    masks = _build_masks().astype(ml_dtypes.bfloat16)
    hb = np.ascontiguousarray((0.5 * b_out).reshape(1, DM))
    in_maps = []
    for c in range(NCORES):
        b, g = c // 2, c % 2
        in_maps.append(
            {
                "xT": xT[b],
                "wq": np.ascontiguousarray(w_qkv[:, LI * g : LI * (g + 1)]),
                "wk": np.ascontiguousarray(w_qkv[:, DM + LI * g : DM + LI * (g + 1)]),
                "wv": np.ascontiguousarray(
                    w_qkv[:, 2 * DM + LI * g : 2 * DM + LI * (g + 1)]
                ),
                "wo": np.ascontiguousarray(w_out[LI * g : LI * (g + 1), :]),
                "hb": hb,
                "mask": masks,
            }
        )
    return in_maps


def _assemble(results):
    y = np.empty((B, N, DM), np.float32)
    for c in range(NCORES):
        b, g = c // 2, c % 2
        o = results[c]["out"]  # [1024, 1024] of token stripes
        for ch in range(NCHUNK - 1):
            t0 = ch * CHUNK + g * 256
            y[b, t0 : t0 + 256] = o[ch * 256 : (ch + 1) * 256]
        ch = NCHUNK - 1  # last chunk: two half-size ReduceScatter pieces
        for p in range(2):
            t0 = ch * CHUNK + p * 256 + g * 128
            r0 = ch * 256 + p * 128
            y[b, t0 : t0 + 128] = o[r0 : r0 + 128]
    return y


def _install_ntff_hook_shim():
    """The container's antenv package lacks axon_hooks; synthesize it so
    run_bass_kernel_spmd(trace=True) can NTFF-profile via the injected .so."""
    import types

    if "antenv.axon_hooks" in sys.modules:
        return
    try:
        from trn_agent_boot.trn_boot import _ntff_profile_via_ctypes

        hook = _ntff_profile_via_ctypes("/opt/axon/libaxon_pjrt.so")
    except Exception as e:  # profiling degrades, run still works
        print(f"ntff hook shim unavailable: {e}")
        hook = None
    mod = types.ModuleType("antenv.axon_hooks")
    _state = {"hook": hook}
    mod.set_axon_ntff_profile_hook = lambda h: _state.__setitem__("hook", h)
    mod.get_axon_ntff_profile_hook = lambda: _state["hook"]
    sys.modules["antenv.axon_hooks"] = mod
    import antenv

    antenv.axon_hooks = mod


def _run(in_maps, trace=False):
    from concourse import bass_utils

    if trace:
        _install_ntff_hook_shim()
    nc = _get_graph()
    return bass_utils.run_bass_kernel_spmd(
        nc, in_maps, core_ids=list(range(NCORES)), trace=trace
    )


def kernel(x, w_qkv, w_out, b_out):
    res = _run(_make_in_maps(x, w_qkv, w_out, b_out), trace=False)
    return _assemble(res.results)


def kernel_timed(x, w_qkv, w_out, b_out):
    res = _run(_make_in_maps(x, w_qkv, w_out, b_out), trace=True)
    return _assemble(res.results), res



# revision 4
# speedup vs baseline: 1.1107x; 1.1107x over previous
"""Trainium2 distributed causal attention kernel (8 NeuronCores).

Problem: x[4,2048,1024] -> qkv proj -> 16-head causal attention -> out proj.

Sharding (uniform SPMD graph on all 8 cores):
  core c = (batch b = c//2, head-group g = c%2 of 8 heads).
  Each core: projects q/k/v for its 8 heads over the full 2048 tokens of its
  batch, runs causal flash-style attention (no max subtraction -- scores are
  O(1) for this input distribution), computes the partial output projection
  with its 512 inner dims of w_out, adds b_out/2, then a pairwise
  ReduceScatter(add, bf16) over {2b, 2b+1} yields final output token-stripes.
  Host reassembles stripes. No other collectives.

v2 (all-bf16, stall fixes over the f32r baseline):
  - x / w_qkv / w_out / b_out are converted to bf16 on the host; every matmul
    runs at the bf16 streaming rate.  Weight tiles are persistent in SBUF and
    DMA-ordered so the first projection group's operands land first.
  - diagonal-chunk key blocks only compute the un-masked column range
    (col0 = v*128 trimming on QK matmul, exp, and PV matmul); the pt zero
    memsets disappear.
  - out-proj bias is added by the DVE during psum evacuation against a
    DMA-broadcast [128,1024] bias tile instead of a ones-row matmul.
  - softmax epilogue is per-head-pair: reciprocal of the denominator row is
    taken straight out of the PV psum and its [64,512] broadcast DMA issues
    while the next head-pair is still computing; the chunk epilogue is just
    8 DVE multiplies.
  - chunks are processed in descending cost order (3,2,1,0) and the final
    chunk's out-proj ReduceScatters in four 128-token pieces (bf16), with the
    RS-dependent output stores issued on the gpsimd queue so they cannot
    head-of-line-block the sync DMA queue.
"""

import sys

sys.path.insert(0, "/opt/trn_rl_repo")

import numpy as np

B, N, DM = 4, 2048, 1024
H, DH = 16, 64
HG = 8  # heads per core
LI = HG * DH  # local inner = 512
NCORES = 8
CHUNK = 512  # q-chunk tokens
NCHUNK = N // CHUNK  # 4
KB = 128  # k-block size
VW = DH + 1  # v columns per head incl. ones column
LAST_NRS = 4  # RS split of the final processed chunk (chunk 0)

_GRAPH = None


def _build_graph():
    from concourse import bacc, bass, mybir, tile

    f32 = mybir.dt.float32
    bf16 = mybir.dt.bfloat16
    Exp = mybir.ActivationFunctionType.Exp

    nc = bacc.Bacc("TRN2", target_bir_lowering=False, debug=False)

    xT_d = nc.dram_tensor("xT", [DM, N], bf16, kind="ExternalInput")
    wq_d = nc.dram_tensor("wq", [DM, LI], bf16, kind="ExternalInput")
    wk_d = nc.dram_tensor("wk", [DM, LI], bf16, kind="ExternalInput")
    wv_d = nc.dram_tensor("wv", [DM, LI], bf16, kind="ExternalInput")
    wo_d = nc.dram_tensor("wo", [LI, DM], bf16, kind="ExternalInput")
    hb_d = nc.dram_tensor("hb", [1, DM], bf16, kind="ExternalInput")
    mask_d = nc.dram_tensor("mask", [KB, KB], bf16, kind="ExternalInput")
    out_d = nc.dram_tensor("out", [N // 2, DM], bf16, kind="ExternalOutput")

    RG = [[0, 1], [2, 3], [4, 5], [6, 7]]

    with tile.TileContext(nc) as tc:
        with (
            tc.tile_pool(name="persist", bufs=1) as pers,
            tc.tile_pool(name="xpool", bufs=1) as xpool,
            tc.tile_pool(name="work", bufs=4) as work,
            tc.tile_pool(name="mmps", bufs=2, space="PSUM") as mmps,
            tc.tile_pool(name="simps", bufs=4, space="PSUM") as simps,
            tc.tile_pool(name="pvps", bufs=2, space="PSUM") as pvps,
            tc.tile_pool(name="dram", bufs=2, space="DRAM") as dram,
        ):
            # ---- persistent weights / constants; DMA order matters: the
            # first kq-projection group needs wk + xT token-chunk 0 first ----
            wkt = [pers.tile([128, LI], bf16, tag=f"wk{d}", name=f"wk{d}") for d in range(8)]
            for d in range(8):
                nc.sync.dma_start(out=wkt[d][:, :], in_=wk_d[d * 128 : (d + 1) * 128, :])

            xTc = [[None] * 4 for _ in range(8)]
            for cc in range(4):
                for d in range(8):
                    t = xpool.tile([128, CHUNK], bf16, tag=f"x{d}_{cc}")
                    nc.sync.dma_start(
                        out=t[:, :],
                        in_=xT_d[d * 128 : (d + 1) * 128, cc * 512 : (cc + 1) * 512],
                    )
                    xTc[d][cc] = t
                if cc == 0:
                    wqt = [pers.tile([128, LI], bf16, tag=f"wq{d}", name=f"wq{d}") for d in range(8)]
                    for d in range(8):
                        nc.sync.dma_start(
                            out=wqt[d][:, :], in_=wq_d[d * 128 : (d + 1) * 128, :]
                        )
                if cc == 1:
                    wvt = [pers.tile([128, LI], bf16, tag=f"wv{d}", name=f"wv{d}") for d in range(8)]
                    for d in range(8):
                        nc.sync.dma_start(
                            out=wvt[d][:, :], in_=wv_d[d * 128 : (d + 1) * 128, :]
                        )

            mask_sb = pers.tile([KB, KB], bf16, tag="mask")
            nc.sync.dma_start(out=mask_sb[:, :], in_=mask_d[:, :])

            wo_bf = []
            for it in range(4):
                wob = pers.tile([128, DM], bf16, tag=f"wo{it}")
                nc.sync.dma_start(out=wob[:, :], in_=wo_d[it * 128 : (it + 1) * 128, :])
                wo_bf.append(wob)

            hb_sb = pers.tile([1, DM], bf16, tag="hb")
            nc.sync.dma_start(out=hb_sb[:, :], in_=hb_d[:, :])
            hbb = pers.tile([128, DM], bf16, tag="hbb")
            hrow = hb_sb[0:1, :]
            hsrc = bass.AP(
                tensor=hrow.tensor,
                offset=hrow.offset,
                ap=[[DM, 1], [0, 128], [1, DM]],
            )
            nc.sync.dma_start(out=hbb[:, :], in_=hsrc)

            # ---- phase 1: projections (all bf16) ----
            kT = [pers.tile([128, N], bf16, tag=f"kT{i}", name=f"kT{i}") for i in range(4)]
            qT = [pers.tile([128, N], bf16, tag=f"qT{i}", name=f"qT{i}") for i in range(4)]

            for wt, dst in ((wkt, kT), (wqt, qT)):
                for tt in range(4):
                    for it in range(4):
                        ps = mmps.tile([128, 512], f32, tag="mm")
                        for d in range(8):
                            nc.tensor.matmul(
                                ps[:, :],
                                lhsT=wt[d][:, it * 128 : (it + 1) * 128],
                                rhs=xTc[d][tt][:, :],
                                start=(d == 0),
                                stop=(d == 7),
                            )
                        nc.vector.tensor_copy(
                            dst[it][:, tt * 512 : (tt + 1) * 512], ps[:, :]
                        )

            v_aug = [pers.tile([128, HG * VW], bf16, tag=f"va{t}", name=f"va{t}") for t in range(16)]
            for tt in range(16):
                va3 = v_aug[tt].rearrange("p (h c) -> p h c", h=HG)
                nc.vector.memset(va3[:, :, DH : DH + 1], 1.0)
                ps = mmps.tile([128, 512], f32, tag="mm")
                for d in range(8):
                    nc.tensor.matmul(
                        ps[:, :],
                        lhsT=xTc[d][tt // 4][:, (tt % 4) * 128 : (tt % 4 + 1) * 128],
                        rhs=wvt[d][:, :],
                        start=(d == 0),
                        stop=(d == 7),
                    )
                nc.vector.tensor_copy(
                    va3[:, :, 0:DH], ps.rearrange("p (h c) -> p h c", h=HG)
                )

            # ---- phases 2+3: attention + out-proj + RS, chunk-pipelined,
            # descending chunk order so the smallest chunk's out-proj is the
            # kernel tail ----
            chunk_state = {}

            def attention_chunk(c):
                nk = 4 * (c + 1)
                vals = [None] * 8
                rbs = [None] * 8
                for hp in range(4):
                    pvs = [
                        pvps.tile([VW, CHUNK], f32, tag="pv", name="pv")
                        for _ in range(2)
                    ]
                    sims_of = {}

                    def col0_of(jb):
                        v = jb - (nk - 4)
                        return max(0, v) * KB, v

                    def qk_step(jb):
                        col0, _ = col0_of(jb)
                        sims = [
                            simps.tile([128, CHUNK], f32, tag="sim", name="sim")
                            for _ in range(2)
                        ]
                        for e in range(2):
                            nc.tensor.matmul(
                                sims[e][:, col0:CHUNK],
                                lhsT=kT[hp][
                                    64 * e : 64 * e + 64, jb * KB : (jb + 1) * KB
                                ],
                                rhs=qT[hp][
                                    64 * e : 64 * e + 64,
                                    c * CHUNK + col0 : (c + 1) * CHUNK,
                                ],
                                start=True,
                                stop=True,
                            )
                        sims_of[jb] = sims

                    def pv_step(jb):
                        sims = sims_of.pop(jb)
                        col0, v = col0_of(jb)
                        for e in range(2):
                            h = 2 * hp + e
                            pt = work.tile([128, CHUNK], bf16, tag=f"pt{e}", bufs=3)
                            nc.scalar.activation(
                                pt[:, col0:CHUNK],
                                sims[e][:, col0:CHUNK],
                                Exp,
                                scale=float(DH**-0.5),
                            )
                            if v >= 0:
                                nc.vector.tensor_mul(
                                    pt[:, col0 : col0 + KB],
                                    pt[:, col0 : col0 + KB],
                                    mask_sb[:, :],
                                )
                            nc.tensor.matmul(
                                pvs[e][:, col0:CHUNK],
                                lhsT=v_aug[jb][:, h * VW : (h + 1) * VW],
                                rhs=pt[:, col0:CHUNK],
                                start=(jb == 0),
                                stop=(jb == nk - 1),
                            )

                    qk_step(0)
                    for jb in range(1, nk):
                        qk_step(jb)
                        pv_step(jb - 1)
                    pv_step(nk - 1)

                    # per-head-pair epilogue front half: denominators straight
                    # out of psum, broadcast DMA in flight during the next
                    # head-pair; psum rows 0:64 evacuate to sbuf.
                    for e in range(2):
                        h = 2 * hp + e
                        rc1 = work.tile([1, CHUNK], f32, tag=f"rc{e}", bufs=2)
                        nc.vector.reciprocal(rc1[:, :], pvs[e][DH : DH + 1, :])
                        rcb1 = work.tile([1, CHUNK], bf16, tag=f"rcb{e}", bufs=2)
                        nc.vector.tensor_copy(rcb1[:, :], rc1[:, :])
                        rb = work.tile([DH, CHUNK], bf16, tag=f"rb{h}", bufs=2)
                        rrow = rcb1[0:1, :]
                        rsrc = bass.AP(
                            tensor=rrow.tensor,
                            offset=rrow.offset,
                            ap=[[CHUNK, 1], [0, DH], [1, CHUNK]],
                        )
                        nc.sync.dma_start(out=rb[:, :], in_=rsrc)
                        rbs[h] = rb
                        t = work.tile(
                            [DH, CHUNK], bf16, tag=f"pvsb{h}", bufs=2, name=f"pvsb{h}"
                        )
                        nc.vector.tensor_copy(t[:, :], pvs[e][0:DH, :])
                        vals[h] = t

                chunk_state[c] = (vals, rbs)

            def epilogue_chunk(c):
                vals, rbs = chunk_state[c]
                aos = [
                    work.tile([128, CHUNK], bf16, tag=f"ao{i}", name=f"ao{i}", bufs=2)
                    for i in range(4)
                ]
                for h in range(8):
                    nc.vector.tensor_mul(
                        aos[h // 2][64 * (h % 2) : 64 * (h % 2) + 64, :],
                        vals[h][:, :],
                        rbs[h][:, :],
                    )
                chunk_state[c] = aos

            def outproj_chunk(c, n_rs=1):
                aos = chunk_state.pop(c)
                pd = dram.tile([CHUNK, DM], bf16, tag="pd")
                ts_per_rs = 4 // n_rs
                for rs_i in range(n_rs):
                    for ts in range(rs_i * ts_per_rs, (rs_i + 1) * ts_per_rs):
                        for ct in range(2):
                            po = mmps.tile([128, 512], f32, tag="mm")
                            for it in range(4):
                                nc.tensor.matmul(
                                    po[:, :],
                                    lhsT=aos[it][:, ts * 128 : (ts + 1) * 128],
                                    rhs=wo_bf[it][:, ct * 512 : (ct + 1) * 512],
                                    start=(it == 0),
                                    stop=(it == 3),
                                )
                            ob = work.tile([128, 512], bf16, tag="ob", name="ob", bufs=2)
                            nc.vector.tensor_add(
                                ob[:, :], po[:, :], hbb[:, ct * 512 : (ct + 1) * 512]
                            )
                            nc.sync.dma_start(
                                out=pd[
                                    ts * 128 : (ts + 1) * 128,
                                    ct * 512 : (ct + 1) * 512,
                                ],
                                in_=ob[:, :],
                            )
                    rows = CHUNK // n_rs
                    rs = dram.tile(
                        [rows // 2, DM],
                        bf16,
                        tag="rs",
                        name="rs",
                        padded_shape=[CHUNK // 2, DM],
                    )
                    nc.gpsimd.collective_compute(
                        "ReduceScatter",
                        mybir.AluOpType.add,
                        replica_groups=RG,
                        ins=[pd[rs_i * rows : (rs_i + 1) * rows, :].opt()],
                        outs=[rs[:, :].opt()],
                    )
                    out_r0 = c * 256 + rs_i * (rows // 2)
                    nc.gpsimd.dma_start(
                        out=out_d[out_r0 : out_r0 + rows // 2, :], in_=rs[:, :]
                    )

            # schedule: att(3) epi(3) att(2) out(3) epi(2) att(1) out(2)
            #           epi(1) att(0) out(1) epi(0) out(0 split)
            attention_chunk(3)
            for c in (2, 1, 0):
                epilogue_chunk(c + 1)
                attention_chunk(c)
                outproj_chunk(c + 1)
            epilogue_chunk(0)
            outproj_chunk(0, n_rs=LAST_NRS)

    nc.finalize()
    return nc


def _get_graph():
    global _GRAPH
    if _GRAPH is None:
        _GRAPH = _build_graph()
    return _GRAPH


def _build_masks():
    # [j, ti] = 1 where ti >= j: token ti attends key j within the diagonal block
    return np.ascontiguousarray(np.triu(np.ones((KB, KB), np.float32)))


def _make_in_maps(x, w_qkv, w_out, b_out):
    import ml_dtypes

    bf = ml_dtypes.bfloat16
    x = np.asarray(x, np.float32)
    w_qkv = np.asarray(w_qkv, np.float32).astype(bf)
    w_out = np.asarray(w_out, np.float32).astype(bf)
    b_out = np.asarray(b_out, np.float32)

    xT = [np.ascontiguousarray(x[b].T).astype(bf) for b in range(B)]
    masks = _build_masks().astype(bf)
    hb = np.ascontiguousarray((0.5 * b_out).reshape(1, DM)).astype(bf)
    in_maps = []
    for c in range(NCORES):
        b, g = c // 2, c % 2
        in_maps.append(
            {
                "xT": xT[b],
                "wq": np.ascontiguousarray(w_qkv[:, LI * g : LI * (g + 1)]),
                "wk": np.ascontiguousarray(w_qkv[:, DM + LI * g : DM + LI * (g + 1)]),
                "wv": np.ascontiguousarray(
                    w_qkv[:, 2 * DM + LI * g : 2 * DM + LI * (g + 1)]
                ),
                "wo": np.ascontiguousarray(w_out[LI * g : LI * (g + 1), :]),
                "hb": hb,
                "mask": masks,
            }
        )
    return in_maps


def _assemble(results):
    y = np.empty((B, N, DM), np.float32)
    for c in range(NCORES):
        b, g = c // 2, c % 2
        o = np.asarray(results[c]["out"], np.float32)  # [1024, 1024] token stripes
        for ch in range(NCHUNK):
            n_rs = LAST_NRS if ch == 0 else 1
            rows_per = CHUNK // n_rs
            half = rows_per // 2
            for p in range(n_rs):
                t0 = ch * CHUNK + p * rows_per + g * half
                r0 = ch * 256 + p * half
                y[b, t0 : t0 + half] = o[r0 : r0 + half]
    return y


def _install_ntff_hook_shim():
    """The container's antenv package lacks axon_hooks; synthesize it so
    run_bass_kernel_spmd(trace=True) can NTFF-profile via the injected .so."""
    import types

    if "antenv.axon_hooks" in sys.modules:
        return
    try:
        from trn_agent_boot.trn_boot import _ntff_profile_via_ctypes

        hook = _ntff_profile_via_ctypes("/opt/axon/libaxon_pjrt.so")
    except Exception as e:  # profiling degrades, run still works
        print(f"ntff hook shim unavailable: {e}")
        hook = None
    mod = types.ModuleType("antenv.axon_hooks")
    _state = {"hook": hook}
    mod.set_axon_ntff_profile_hook = lambda h: _state.__setitem__("hook", h)
    mod.get_axon_ntff_profile_hook = lambda: _state["hook"]
    sys.modules["antenv.axon_hooks"] = mod
    import antenv

    antenv.axon_hooks = mod


def _run(in_maps, trace=False):
    from concourse import bass_utils

    if trace:
        _install_ntff_hook_shim()
    nc = _get_graph()
    return bass_utils.run_bass_kernel_spmd(
        nc, in_maps, core_ids=list(range(NCORES)), trace=trace
    )


def kernel(x, w_qkv, w_out, b_out):
    res = _run(_make_in_maps(x, w_qkv, w_out, b_out), trace=False)
    return _assemble(res.results)


def kernel_timed(x, w_qkv, w_out, b_out):
    res = _run(_make_in_maps(x, w_qkv, w_out, b_out), trace=True)
    return _assemble(res.results), res


# revision 6
# speedup vs baseline: 1.3181x; 1.1868x over previous
"""Trainium2 distributed causal attention kernel (8 NeuronCores).

Problem: x[4,2048,1024] -> qkv proj -> 16-head causal attention -> out proj.

Sharding (uniform SPMD graph on all 8 cores):
  core c = (batch b = c//2, head-group g = c%2 of 8 heads).
  Each core: projects q/k/v for its 8 heads over the full 2048 tokens of its
  batch, runs causal flash-style attention (no max subtraction -- scores are
  O(1) for this input distribution), computes the partial output projection
  with its 512 inner dims of w_out, adds b_out/2, then a pairwise
  ReduceScatter(add, bf16) over {2b, 2b+1} yields final output token-stripes.
  Host reassembles stripes. No other collectives.

v2 (all-bf16, stall fixes over the f32r baseline):
  - x / w_qkv / w_out / b_out are converted to bf16 on the host; every matmul
    runs at the bf16 streaming rate.  Weight tiles are persistent in SBUF and
    DMA-ordered so the first projection group's operands land first.
  - diagonal-chunk key blocks only compute the un-masked column range
    (col0 = v*128 trimming on QK matmul, exp, and PV matmul); the pt zero
    memsets disappear.
  - out-proj bias is added by the DVE during psum evacuation against a
    DMA-broadcast [128,1024] bias tile instead of a ones-row matmul.
  - softmax epilogue is per-head-pair: reciprocal of the denominator row is
    taken straight out of the PV psum and its [64,512] broadcast DMA issues
    while the next head-pair is still computing; the chunk epilogue is just
    8 DVE multiplies.
  - chunks are processed in descending cost order (3,2,1,0) and the final
    chunk's out-proj ReduceScatters in four 128-token pieces (bf16), with the
    RS-dependent output stores issued on the gpsimd queue so they cannot
    head-of-line-block the sync DMA queue.
"""

import sys

sys.path.insert(0, "/opt/trn_rl_repo")

import numpy as np

B, N, DM = 4, 2048, 1024
H, DH = 16, 64
HG = 8  # heads per core
LI = HG * DH  # local inner = 512
NCORES = 8
CHUNK = 512  # q-chunk tokens
NCHUNK = N // CHUNK  # 4
KB = 128  # k-block size
VW = DH + 1  # v columns per head incl. ones column
LAST_NRS = 1  # RS split of the final processed chunk (chunk 0); split
# pieces serialize on the CC engine so one piece gives the shortest tail

_GRAPH = None


def _build_graph():
    from concourse import bacc, bass, mybir, tile

    f32 = mybir.dt.float32
    bf16 = mybir.dt.bfloat16
    Exp = mybir.ActivationFunctionType.Exp

    nc = bacc.Bacc("TRN2", target_bir_lowering=False, debug=False)

    xT_d = nc.dram_tensor("xT", [DM, N], bf16, kind="ExternalInput")
    wq_d = nc.dram_tensor("wq", [DM, LI], bf16, kind="ExternalInput")
    wk_d = nc.dram_tensor("wk", [DM, LI], bf16, kind="ExternalInput")
    wv_d = nc.dram_tensor("wv", [DM, LI], bf16, kind="ExternalInput")
    wo_d = nc.dram_tensor("wo", [LI, DM], bf16, kind="ExternalInput")
    hb_d = nc.dram_tensor("hb", [1, DM], bf16, kind="ExternalInput")
    mask_d = nc.dram_tensor("mask", [KB, KB], bf16, kind="ExternalInput")
    out_d = nc.dram_tensor("out", [N // 2, DM], bf16, kind="ExternalOutput")

    RG = [[0, 1], [2, 3], [4, 5], [6, 7]]

    with tile.TileContext(nc) as tc:
        with (
            tc.tile_pool(name="persist", bufs=1) as pers,
            tc.tile_pool(name="xpool", bufs=1) as xpool,
            tc.tile_pool(name="work", bufs=4) as work,
            tc.tile_pool(name="mmps", bufs=2, space="PSUM") as mmps,
            tc.tile_pool(name="simps", bufs=4, space="PSUM") as simps,
            tc.tile_pool(name="pvps", bufs=2, space="PSUM") as pvps,
            tc.tile_pool(name="dram", bufs=2, space="DRAM") as dram,
        ):
            # ---- persistent weights / constants; DMA order matters: the
            # first kq-projection group needs wk + xT token-chunk 0 first ----
            wkt = [pers.tile([128, LI], bf16, tag=f"wk{d}", name=f"wk{d}") for d in range(8)]
            for d in range(8):
                nc.sync.dma_start(out=wkt[d][:, :], in_=wk_d[d * 128 : (d + 1) * 128, :])

            xTc = [[None] * 4 for _ in range(8)]
            for cc in range(4):
                for d in range(8):
                    t = xpool.tile([128, CHUNK], bf16, tag=f"x{d}_{cc}")
                    nc.sync.dma_start(
                        out=t[:, :],
                        in_=xT_d[d * 128 : (d + 1) * 128, cc * 512 : (cc + 1) * 512],
                    )
                    xTc[d][cc] = t
                if cc == 0:
                    wqt = [pers.tile([128, LI], bf16, tag=f"wq{d}", name=f"wq{d}") for d in range(8)]
                    for d in range(8):
                        nc.sync.dma_start(
                            out=wqt[d][:, :], in_=wq_d[d * 128 : (d + 1) * 128, :]
                        )
                if cc == 1:
                    wvt = [pers.tile([128, LI], bf16, tag=f"wv{d}", name=f"wv{d}") for d in range(8)]
                    for d in range(8):
                        nc.sync.dma_start(
                            out=wvt[d][:, :], in_=wv_d[d * 128 : (d + 1) * 128, :]
                        )

            mask_sb = pers.tile([KB, KB], bf16, tag="mask")
            nc.sync.dma_start(out=mask_sb[:, :], in_=mask_d[:, :])

            wo_bf = []
            for it in range(4):
                wob = pers.tile([128, DM], bf16, tag=f"wo{it}")
                nc.sync.dma_start(out=wob[:, :], in_=wo_d[it * 128 : (it + 1) * 128, :])
                wo_bf.append(wob)

            hb_sb = pers.tile([1, DM], bf16, tag="hb")
            nc.sync.dma_start(out=hb_sb[:, :], in_=hb_d[:, :])
            hbb = pers.tile([128, DM], bf16, tag="hbb")
            hrow = hb_sb[0:1, :]
            hsrc = bass.AP(
                tensor=hrow.tensor,
                offset=hrow.offset,
                ap=[[DM, 1], [0, 128], [1, DM]],
            )
            nc.sync.dma_start(out=hbb[:, :], in_=hsrc)

            # ---- phase 1: projections (all bf16) ----
            kT = [pers.tile([128, N], bf16, tag=f"kT{i}", name=f"kT{i}") for i in range(4)]
            qT = [pers.tile([128, N], bf16, tag=f"qT{i}", name=f"qT{i}") for i in range(4)]

            for wt, dst in ((wkt, kT), (wqt, qT)):
                for tt in range(4):
                    for it in range(4):
                        ps = mmps.tile([128, 512], f32, tag="mm")
                        for d in range(8):
                            nc.tensor.matmul(
                                ps[:, :],
                                lhsT=wt[d][:, it * 128 : (it + 1) * 128],
                                rhs=xTc[d][tt][:, :],
                                start=(d == 0),
                                stop=(d == 7),
                            )
                        nc.vector.tensor_copy(
                            dst[it][:, tt * 512 : (tt + 1) * 512], ps[:, :]
                        )

            v_aug = [pers.tile([128, HG * VW], bf16, tag=f"va{t}", name=f"va{t}") for t in range(16)]
            for tt in range(16):
                va3 = v_aug[tt].rearrange("p (h c) -> p h c", h=HG)
                nc.vector.memset(va3[:, :, DH : DH + 1], 1.0)
                ps = mmps.tile([128, 512], f32, tag="mm")
                for d in range(8):
                    nc.tensor.matmul(
                        ps[:, :],
                        lhsT=xTc[d][tt // 4][:, (tt % 4) * 128 : (tt % 4 + 1) * 128],
                        rhs=wvt[d][:, :],
                        start=(d == 0),
                        stop=(d == 7),
                    )
                nc.vector.tensor_copy(
                    va3[:, :, 0:DH], ps.rearrange("p (h c) -> p h c", h=HG)
                )

            # ---- phases 2+3: attention + out-proj + RS, chunk-pipelined,
            # descending chunk order so the smallest chunk's out-proj is the
            # kernel tail ----
            chunk_state = {}

            def attention_chunk(c):
                nk = 4 * (c + 1)
                vals = [None] * 8
                rbs = [None] * 8
                for hp in range(4):
                    pvs = [
                        pvps.tile([VW, CHUNK], f32, tag="pv", name="pv")
                        for _ in range(2)
                    ]
                    sims_of = {}

                    def col0_of(jb):
                        v = jb - (nk - 4)
                        return max(0, v) * KB, v

                    def qk_step(jb):
                        col0, _ = col0_of(jb)
                        sims = [
                            simps.tile([128, CHUNK], f32, tag="sim", name="sim")
                            for _ in range(2)
                        ]
                        for e in range(2):
                            nc.tensor.matmul(
                                sims[e][:, col0:CHUNK],
                                lhsT=kT[hp][
                                    64 * e : 64 * e + 64, jb * KB : (jb + 1) * KB
                                ],
                                rhs=qT[hp][
                                    64 * e : 64 * e + 64,
                                    c * CHUNK + col0 : (c + 1) * CHUNK,
                                ],
                                start=True,
                                stop=True,
                            )
                        sims_of[jb] = sims

                    def pv_step(jb):
                        sims = sims_of.pop(jb)
                        col0, v = col0_of(jb)
                        for e in range(2):
                            h = 2 * hp + e
                            pt = work.tile([128, CHUNK], bf16, tag=f"pt{e}", bufs=3)
                            nc.scalar.activation(
                                pt[:, col0:CHUNK],
                                sims[e][:, col0:CHUNK],
                                Exp,
                                scale=float(DH**-0.5),
                            )
                            if v >= 0:
                                nc.vector.tensor_mul(
                                    pt[:, col0 : col0 + KB],
                                    pt[:, col0 : col0 + KB],
                                    mask_sb[:, :],
                                )
                            nc.tensor.matmul(
                                pvs[e][:, col0:CHUNK],
                                lhsT=v_aug[jb][:, h * VW : (h + 1) * VW],
                                rhs=pt[:, col0:CHUNK],
                                start=(jb == 0),
                                stop=(jb == nk - 1),
                            )

                    qk_step(0)
                    for jb in range(1, nk):
                        qk_step(jb)
                        pv_step(jb - 1)
                    pv_step(nk - 1)

                    # per-head-pair epilogue front half.  Evacuate the psum
                    # FIRST (cheap copies release the WAR hazard on the pv
                    # banks so the next head-pair's matmuls can start), then
                    # run the reciprocal/broadcast chain off the critical
                    # path while the next head-pair computes.
                    dcp = []
                    for e in range(2):
                        h = 2 * hp + e
                        dc = work.tile([1, CHUNK], f32, tag=f"dcp{e}", bufs=2)
                        nc.vector.tensor_copy(dc[:, :], pvs[e][DH : DH + 1, :])
                        dcp.append(dc)
                        t = work.tile(
                            [DH, CHUNK], bf16, tag=f"pvsb{h}", bufs=2, name=f"pvsb{h}"
                        )
                        nc.vector.tensor_copy(t[:, :], pvs[e][0:DH, :])
                        vals[h] = t
                    for e in range(2):
                        h = 2 * hp + e
                        rc1 = work.tile([1, CHUNK], f32, tag=f"rc{e}", bufs=2)
                        nc.vector.reciprocal_approx_fast(rc1[:, :], dcp[e][:, :])
                        rcb1 = work.tile([1, CHUNK], bf16, tag=f"rcb{e}", bufs=2)
                        nc.vector.tensor_copy(rcb1[:, :], rc1[:, :])
                        rb = work.tile([DH, CHUNK], bf16, tag=f"rb{h}", bufs=2)
                        rrow = rcb1[0:1, :]
                        rsrc = bass.AP(
                            tensor=rrow.tensor,
                            offset=rrow.offset,
                            ap=[[CHUNK, 1], [0, DH], [1, CHUNK]],
                        )
                        nc.sync.dma_start(out=rb[:, :], in_=rsrc)
                        rbs[h] = rb

                chunk_state[c] = (vals, rbs)

            def epilogue_chunk(c):
                vals, rbs = chunk_state[c]
                aos = [
                    work.tile([128, CHUNK], bf16, tag=f"ao{i}", name=f"ao{i}", bufs=2)
                    for i in range(4)
                ]
                for h in range(8):
                    nc.vector.tensor_mul(
                        aos[h // 2][64 * (h % 2) : 64 * (h % 2) + 64, :],
                        vals[h][:, :],
                        rbs[h][:, :],
                    )
                chunk_state[c] = aos

            def outproj_chunk(c, n_rs=1):
                aos = chunk_state.pop(c)
                pd = dram.tile([CHUNK, DM], bf16, tag="pd")
                ts_per_rs = 4 // n_rs
                for rs_i in range(n_rs):
                    for ts in range(rs_i * ts_per_rs, (rs_i + 1) * ts_per_rs):
                        for ct in range(2):
                            po = mmps.tile([128, 512], f32, tag="mm")
                            for it in range(4):
                                nc.tensor.matmul(
                                    po[:, :],
                                    lhsT=aos[it][:, ts * 128 : (ts + 1) * 128],
                                    rhs=wo_bf[it][:, ct * 512 : (ct + 1) * 512],
                                    start=(it == 0),
                                    stop=(it == 3),
                                )
                            ob = work.tile([128, 512], bf16, tag="ob", name="ob", bufs=2)
                            nc.vector.tensor_add(
                                ob[:, :], po[:, :], hbb[:, ct * 512 : (ct + 1) * 512]
                            )
                            nc.sync.dma_start(
                                out=pd[
                                    ts * 128 : (ts + 1) * 128,
                                    ct * 512 : (ct + 1) * 512,
                                ],
                                in_=ob[:, :],
                            )
                    rows = CHUNK // n_rs
                    rs = dram.tile(
                        [rows // 2, DM],
                        bf16,
                        tag="rs",
                        name="rs",
                        padded_shape=[CHUNK // 2, DM],
                    )
                    nc.gpsimd.collective_compute(
                        "ReduceScatter",
                        mybir.AluOpType.add,
                        replica_groups=RG,
                        ins=[pd[rs_i * rows : (rs_i + 1) * rows, :].opt()],
                        outs=[rs[:, :].opt()],
                    )
                    out_r0 = c * 256 + rs_i * (rows // 2)
                    nc.gpsimd.dma_start(
                        out=out_d[out_r0 : out_r0 + rows // 2, :], in_=rs[:, :]
                    )

            # schedule: att(3) epi(3) att(2) out(3) epi(2) att(1) out(2)
            #           epi(1) att(0) out(1) epi(0) out(0 split)
            attention_chunk(3)
            for c in (2, 1, 0):
                epilogue_chunk(c + 1)
                attention_chunk(c)
                outproj_chunk(c + 1)
            epilogue_chunk(0)
            outproj_chunk(0, n_rs=LAST_NRS)

    nc.finalize()
    return nc


def _get_graph():
    global _GRAPH
    if _GRAPH is None:
        _GRAPH = _build_graph()
    return _GRAPH


def _build_masks():
    # [j, ti] = 1 where ti >= j: token ti attends key j within the diagonal block
    return np.ascontiguousarray(np.triu(np.ones((KB, KB), np.float32)))


def _make_in_maps(x, w_qkv, w_out, b_out):
    import ml_dtypes

    bf = ml_dtypes.bfloat16
    x = np.asarray(x, np.float32)
    w_qkv = np.asarray(w_qkv, np.float32).astype(bf)
    w_out = np.asarray(w_out, np.float32).astype(bf)
    b_out = np.asarray(b_out, np.float32)

    xT = [np.ascontiguousarray(x[b].T).astype(bf) for b in range(B)]
    masks = _build_masks().astype(bf)
    hb = np.ascontiguousarray((0.5 * b_out).reshape(1, DM)).astype(bf)
    in_maps = []
    for c in range(NCORES):
        b, g = c // 2, c % 2
        in_maps.append(
            {
                "xT": xT[b],
                "wq": np.ascontiguousarray(w_qkv[:, LI * g : LI * (g + 1)]),
                "wk": np.ascontiguousarray(w_qkv[:, DM + LI * g : DM + LI * (g + 1)]),
                "wv": np.ascontiguousarray(
                    w_qkv[:, 2 * DM + LI * g : 2 * DM + LI * (g + 1)]
                ),
                "wo": np.ascontiguousarray(w_out[LI * g : LI * (g + 1), :]),
                "hb": hb,
                "mask": masks,
            }
        )
    return in_maps


def _assemble(results):
    y = np.empty((B, N, DM), np.float32)
    for c in range(NCORES):
        b, g = c // 2, c % 2
        o = np.asarray(results[c]["out"], np.float32)  # [1024, 1024] token stripes
        for ch in range(NCHUNK):
            n_rs = LAST_NRS if ch == 0 else 1
            rows_per = CHUNK // n_rs
            half = rows_per // 2
            for p in range(n_rs):
                t0 = ch * CHUNK + p * rows_per + g * half
                r0 = ch * 256 + p * half
                y[b, t0 : t0 + half] = o[r0 : r0 + half]
    return y


def _install_ntff_hook_shim():
    """The container's antenv package lacks axon_hooks; synthesize it so
    run_bass_kernel_spmd(trace=True) can NTFF-profile via the injected .so."""
    import types

    if "antenv.axon_hooks" in sys.modules:
        return
    try:
        from trn_agent_boot.trn_boot import _ntff_profile_via_ctypes

        hook = _ntff_profile_via_ctypes("/opt/axon/libaxon_pjrt.so")
    except Exception as e:  # profiling degrades, run still works
        print(f"ntff hook shim unavailable: {e}")
        hook = None
    mod = types.ModuleType("antenv.axon_hooks")
    _state = {"hook": hook}
    mod.set_axon_ntff_profile_hook = lambda h: _state.__setitem__("hook", h)
    mod.get_axon_ntff_profile_hook = lambda: _state["hook"]
    sys.modules["antenv.axon_hooks"] = mod
    import antenv

    antenv.axon_hooks = mod


def _run(in_maps, trace=False):
    from concourse import bass_utils

    if trace:
        _install_ntff_hook_shim()
    nc = _get_graph()
    return bass_utils.run_bass_kernel_spmd(
        nc, in_maps, core_ids=list(range(NCORES)), trace=trace
    )


def kernel(x, w_qkv, w_out, b_out):
    res = _run(_make_in_maps(x, w_qkv, w_out, b_out), trace=False)
    return _assemble(res.results)


def kernel_timed(x, w_qkv, w_out, b_out):
    res = _run(_make_in_maps(x, w_qkv, w_out, b_out), trace=True)
    return _assemble(res.results), res
